# revision 1
# baseline (speedup 1.0000x reference)
"""Trainium2 Bass kernel for a single-head attention + FFN transformer block.

Math (per batch b):
  S   = Q @ K^T                      (contraction over d, computed in fp32r)
  E   = exp((S + kbias) * qscale)    kbias[k] = 0 valid / -1e30 masked  (ACT)
                                     qscale[q] = qmask/sqrt(d); invalid q rows
                                     get scale 0 -> exp(0)=1 -> uniform softmax,
                                     exactly matching the reference's fully
                                     masked-row behaviour.
  att = (E @ V) * recip(rowsum)      rowsum from ACT accum_out
  y   = LN1(Q + att)
  H^T = relu(W1^T.T-style matmul)    H^T[o,q] = sum_d W1T[d,o] * yT[d,q]
  Z   = H^T.T @ W2^T                 Z[q,d] = sum_o HT[o,q] * W2T[o,d]
  out = LN2(y + Z)                   (+b2 shift cancels inside LN2 exactly)

Sharding: pure data-parallel, 4 batches per core across 8 NeuronCores.
Host pre-transposes Q/K (and the FFN weights) so all matmul operands load
with the contraction dim on partitions; no collectives needed.
"""

import sys

sys.path.insert(0, "/opt/trn_rl_repo")

import numpy as np

import concourse.bass as bass
import concourse.bacc as bacc
import concourse.mybir as mybir
from concourse import tile
from concourse.bass_utils import run_bass_kernel_spmd

B, QTL, KTL, D = 32, 512, 512, 1024
NCORES = 8
BL = B // NCORES  # batches per core
P = 128
NQT = QTL // P  # 4 q tiles
NKT = KTL // P  # 4 k tiles
NDT = D // P    # 8 d tiles
NCH = D // 512  # 2 free-dim chunks of 512
EPS = 1e-5
NEG = -1.0e30

F32 = mybir.dt.float32
F32R = mybir.dt.float32r
AF = mybir.ActivationFunctionType
ALU = mybir.AluOpType


def _build(apply1: bool, apply2: bool, b1f: float):
    nc = bacc.Bacc(None, target_bir_lowering=False)

    QTd = nc.dram_tensor("QTp", [BL, D, QTL], F32, kind="ExternalInput")
    KTd = nc.dram_tensor("KTp", [BL, D, KTL], F32, kind="ExternalInput")
    Vd = nc.dram_tensor("Vp", [BL, KTL, D], F32, kind="ExternalInput")
    Qd = nc.dram_tensor("Qp", [BL, QTL, D], F32, kind="ExternalInput")
    W1Td = nc.dram_tensor("W1Tp", [D, D], F32, kind="ExternalInput")
    W2Td = nc.dram_tensor("W2Tp", [D, D], F32, kind="ExternalInput")
    QSCd = nc.dram_tensor("QSCp", [BL, P, NQT], F32, kind="ExternalInput")
    KBd = nc.dram_tensor("KBp", [BL, KTL], F32, kind="ExternalInput")
    IDd = nc.dram_tensor("IDp", [P, P], F32, kind="ExternalInput")
    ONEd = nc.dram_tensor("ONEp", [1, P], F32, kind="ExternalInput")
    if apply1:
        G1d = nc.dram_tensor("G1p", [D], F32, kind="ExternalInput")
        B1d = nc.dram_tensor("B1p", [D], F32, kind="ExternalInput")
    if apply2:
        G2d = nc.dram_tensor("G2p", [D], F32, kind="ExternalInput")
        B2d = nc.dram_tensor("B2p", [D], F32, kind="ExternalInput")
    OUTd = nc.dram_tensor("OUTp", [BL, QTL, D], F32, kind="ExternalOutput")

    with tile.TileContext(nc) as tc:
        with (
            tc.tile_pool(name="const", bufs=1) as pc,
            tc.tile_pool(name="wts", bufs=1) as pw,
            tc.tile_pool(name="qkin", bufs=1) as pin,
            tc.tile_pool(name="mid", bufs=1) as pmid,
            tc.tile_pool(name="stream", bufs=2) as pst,
            tc.tile_pool(name="small", bufs=2) as psm,
            tc.tile_pool(name="psS", bufs=1, space="PSUM") as psS,
            tc.tile_pool(name="psT", bufs=2, space="PSUM") as psT,
            tc.tile_pool(name="psU", bufs=2, space="PSUM") as psU,
        ):
            for b in range(BL):
                # --- per-batch loads ---
                if b == 0:
                    identr = pc.tile([P, P], F32R, name="identr")
                    nc.sync.dma_start(identr, IDd[:, :].bitcast(F32R))
                QTs = pin.tile([P, NDT, QTL], F32R, tag="qts")
                KTs = pin.tile([P, NDT, KTL], F32R, tag="kts")
                qtr = QTd[b].rearrange("(t p) q -> p t q", p=P).bitcast(F32R)
                ktr = KTd[b].rearrange("(t p) k -> p t k", p=P).bitcast(F32R)
                npc = 4 if b == 0 else 2
                hd = NDT // npc
                for hh in range(npc):
                    sl = slice(hh * hd, (hh + 1) * hd)
                    nc.sync.dma_start(QTs[:, sl, :], qtr[:, sl, :])
                    nc.sync.dma_start(KTs[:, sl, :], ktr[:, sl, :])
                qsc = psm.tile([P, NQT], F32, tag="qsc")
                nc.sync.dma_start(qsc, QSCd[b])
                kbr = pin.tile([1, KTL], F32R, tag="kbr")
                nc.sync.dma_start(kbr, KBd[b][None, :].bitcast(F32R))
                if b == 0:
                    # --- one-time constants / weights ---
                    zb = pc.tile([P, 1], F32)
                    nc.vector.memset(zb, 0.0)
                    epsb = pc.tile([P, 1], F32)
                    nc.vector.memset(epsb, EPS)
                    b1b = pc.tile([P, 1], F32)
                    nc.vector.memset(b1b, b1f)
                    onesr = pc.tile([1, P], F32R)
                    nc.sync.dma_start(onesr, ONEd[:, :].bitcast(F32R))
                    W1Ts = pw.tile([P, NDT, D], F32R)
                    W2Ts = pw.tile([P, NDT, D], F32R)
                    if apply1:
                        g1t = pc.tile([P, D], F32)
                        nc.gpsimd.dma_start(
                            g1t,
                            bass.AP(tensor=G1d, offset=0, ap=[[0, P], [1, D]]),
                        )
                        b1t = pc.tile([P, D], F32)
                        nc.gpsimd.dma_start(
                            b1t,
                            bass.AP(tensor=B1d, offset=0, ap=[[0, P], [1, D]]),
                        )
                    if apply2:
                        g2t = pc.tile([P, D], F32)
                        nc.gpsimd.dma_start(
                            g2t,
                            bass.AP(tensor=G2d, offset=0, ap=[[0, P], [1, D]]),
                        )
                        b2t = pc.tile([P, D], F32)
                        nc.gpsimd.dma_start(
                            b2t,
                            bass.AP(tensor=B2d, offset=0, ap=[[0, P], [1, D]]),
                        )
                Vs = pin.tile([P, NKT, D], F32R, tag="vs")
                vr = Vd[b].rearrange("(t p) d -> p t d", p=P).bitcast(F32R)
                nc.sync.dma_start(Vs, vr)
                qps = []
                for hh in range(2):
                    qp = pst.tile([P, 2, D], F32, tag="qn", bufs=2,
                                  name=f"qn{hh}")
                    nc.sync.dma_start(
                        qp,
                        Qd[b].rearrange("(t p) d -> p t d", p=P)[
                            :, 2 * hh : 2 * hh + 2, :
                        ],
                    )
                    qps.append(qp)

                ET = pmid.tile([P, NDT, QTL], F32R, tag="eh")
                rowsum = psm.tile([P, NQT], F32, tag="rowsum")
                recip = psm.tile([P, NQT], F32, tag="recip")

                if b == 0:
                    # Warmup: keep the PE activity monitor busy with identity
                    # matmuls while batch-0's inputs stream in, so the real
                    # matmuls start at full clock.
                    wps = psT.tile([P, NKT, P], F32R, tag="tr", name="warm")
                    for _ in range(100):
                        nc.tensor.matmul(
                            wps[:, 0, :].bitcast(F32), identr, identr, start=True, stop=True
                        )

                # --- S = Q K^T (+kbias): dt-outer so all 4 PSUM banks
                # accumulate in parallel and PE streams off the first
                # arriving QT/KT chunk ---
                Sps = [psS.tile([P, KTL], F32, tag=f"s{qt}", name=f"sps{qt}") for qt in range(NQT)]
                for dt in range(NDT):
                    for qt in range(NQT):
                        nc.tensor.matmul(
                            Sps[qt],
                            QTs[:, dt, qt * P : (qt + 1) * P],
                            KTs[:, dt, :],
                            start=(dt == 0),
                            stop=False,
                        )
                Es = []
                for qt in range(NQT):
                    nc.tensor.matmul(
                        Sps[qt], onesr[:, :], kbr[:, :], start=False, stop=True
                    )
                    E = pst.tile([P, KTL], F32R, tag=f"e{qt % 2}")
                    nc.scalar.activation(
                        E,
                        Sps[qt],
                        AF.Exp,
                        bias=zb[:, :],
                        scale=qsc[:, qt : qt + 1],
                        accum_out=rowsum[:, qt : qt + 1],
                    )
                    Es.append(E)
                    nc.vector.reciprocal(
                        recip[:, qt : qt + 1], rowsum[:, qt : qt + 1]
                    )
                    if b == 0 and qt == 0:
                        # Weight loads issued on the ACT DMA path after the
                        # first exp so they drain behind batch-0's inputs.
                        nc.scalar.dma_start(
                            W1Ts,
                            W1Td.rearrange("(t p) o -> p t o", p=P).bitcast(F32R),
                        )
                        nc.scalar.dma_start(
                            W2Ts,
                            W2Td.rearrange("(t p) o -> p t o", p=P).bitcast(F32R),
                        )

                # --- E^T: 4 transposes into one PSUM bank, 1 copy per qt ---
                for qt in range(NQT):
                    tps = psT.tile([P, NKT, P], F32R, tag="tr")
                    for kt in range(NKT):
                        nc.tensor.transpose(
                            tps[:, kt, :], Es[qt][:, kt * P : (kt + 1) * P],
                            identr,
                        )
                    nc.scalar.copy(ET[:, 0:NKT, qt * P : (qt + 1) * P], tps)

                # --- U = E V, att, residual, LN1 ---
                y = pmid.tile([P, NQT, D], F32R, tag="y")
                YT = pmid.tile([P, NDT, QTL], F32R, tag="yt")

                def emit_ytr(qt):
                    # y^T: 4 transposes per PSUM bank, 2 copies per q tile;
                    # staggered between U matmul groups so real matmuls keep
                    # feeding the PE activity monitor.
                    for half in range(2):
                        tps = psT.tile([P, NKT, P], F32R, tag="tr", name="tps")
                        for j in range(4):
                            dt = half * 4 + j
                            nc.tensor.transpose(
                                tps[:, j, :], y[:, qt, dt * P : (dt + 1) * P],
                                identr,
                            )
                        nc.scalar.copy(
                            YT[:, half * 4 : half * 4 + 4, qt * P : (qt + 1) * P],
                            tps,
                        )
                qres_l = []
                for qt in range(NQT):
                    qres = pst.tile([P, D], F32, tag="big4", bufs=3)
                    qres_l.append(qres)
                    for ch in range(NCH):
                        Ups = psU.tile([P, 512], F32, tag="u")
                        for kt in range(NKT):
                            nc.tensor.matmul(
                                Ups,
                                ET[:, kt, qt * P : (qt + 1) * P],
                                Vs[:, kt, ch * 512 : (ch + 1) * 512],
                                start=(kt == 0),
                                stop=(kt == NKT - 1),
                            )
                        qch = qres[:, ch * 512 : (ch + 1) * 512]
                        nc.scalar.activation(
                            qch, Ups, AF.Copy, bias=0.0,
                            scale=recip[:, qt : qt + 1],
                        )
                        nc.vector.tensor_add(
                            qch, qch,
                            qps[qt // 2][:, qt % 2, ch * 512 : (ch + 1) * 512],
                        )
                    stats = psm.tile([P, NCH, 6], F32, tag="st1")
                    for ch in range(NCH):
                        nc.vector.bn_stats(
                            stats[:, ch, :], qres[:, ch * 512 : (ch + 1) * 512]
                        )
                    mv = psm.tile([P, 2], F32, tag="mv1")
                    nc.vector.bn_aggr(mv, stats)
                    std = psm.tile([P, 1], F32, tag="std1")
                    nc.scalar.activation(std, mv[:, 1:2], AF.Sqrt, bias=epsb[:, :])
                    nc.vector.reciprocal(std, std)
                    nc.vector.tensor_scalar(
                        y[:, qt, :],
                        qres,
                        scalar1=mv[:, 0:1],
                        scalar2=std,
                        op0=ALU.subtract,
                        op1=ALU.mult,
                    )
                    if apply1:
                        yf = y[:, qt, :].bitcast(F32)
                        nc.vector.tensor_mul(yf, yf, g1t)
                        nc.vector.tensor_add(yf, yf, b1t)
                    if qt >= 2:
                        emit_ytr(qt - 2)
                emit_ytr(NQT - 2)
                emit_ytr(NQT - 1)


                # --- FFN1: H^T = relu(sum_d W1T[d,o] yT[d,q]) ---
                HT = pmid.tile([P, NDT, QTL], F32R, tag="eh")
                for ot in range(NDT):
                    Hps = psU.tile([P, QTL], F32, tag="u")
                    for dt in range(NDT):
                        nc.tensor.matmul(
                            Hps,
                            W1Ts[:, dt, ot * P : (ot + 1) * P],
                            YT[:, dt, :],
                            start=(dt == 0),
                            stop=(dt == NDT - 1),
                        )
                    nc.scalar.activation(HT[:, ot, :], Hps, AF.Relu, bias=b1b[:, :])

                # --- FFN2 + LN2 + store ---
                for qt in range(NQT):
                    r2 = pst.tile([P, D], F32, tag="big4", bufs=3)
                    for ch in range(NCH):
                        Zps = psU.tile([P, 512], F32, tag="u")
                        for ot in range(NDT):
                            nc.tensor.matmul(
                                Zps,
                                HT[:, ot, qt * P : (qt + 1) * P],
                                W2Ts[:, ot, ch * 512 : (ch + 1) * 512],
                                start=(ot == 0),
                                stop=(ot == NDT - 1),
                            )
                        nc.vector.tensor_add(
                            r2[:, ch * 512 : (ch + 1) * 512],
                            Zps,
                            y[:, qt, ch * 512 : (ch + 1) * 512].bitcast(F32),
                        )
                    stats2 = psm.tile([P, NCH, 6], F32, tag="st2")
                    for ch in range(NCH):
                        nc.vector.bn_stats(
                            stats2[:, ch, :], r2[:, ch * 512 : (ch + 1) * 512]
                        )
                    mv2 = psm.tile([P, 2], F32, tag="mv2")
                    nc.vector.bn_aggr(mv2, stats2)
                    std2 = psm.tile([P, 1], F32, tag="std2")
                    nc.scalar.activation(std2, mv2[:, 1:2], AF.Sqrt, bias=epsb[:, :])
                    nc.vector.reciprocal(std2, std2)
                    stg = pst.tile([P, D], F32, tag="stg", bufs=2)
                    nc.vector.tensor_scalar(
                        stg,
                        r2,
                        scalar1=mv2[:, 0:1],
                        scalar2=std2,
                        op0=ALU.subtract,
                        op1=ALU.mult,
                    )
                    if apply2:
                        nc.vector.tensor_mul(stg, stg, g2t)
                        nc.vector.tensor_add(stg, stg, b2t)
                    nc.gpsimd.dma_start(
                        OUTd[b].rearrange("(t p) d -> p t d", p=P)[:, qt, :],
                        stg,
                    )

    nc.finalize()
    return nc


def _prepare(Q, K, V, Q_lengths, K_lengths, W1, b1, W2, b2,
             ln1_g, ln1_b, ln2_g, ln2_b):
    Q = np.asarray(Q, dtype=np.float32)
    K = np.asarray(K, dtype=np.float32)
    V = np.asarray(V, dtype=np.float32)
    W1 = np.asarray(W1, dtype=np.float32)
    W2 = np.asarray(W2, dtype=np.float32)
    qlen = np.asarray(Q_lengths).astype(np.int64)
    klen = np.asarray(K_lengths).astype(np.int64)
    g1 = np.asarray(ln1_g, dtype=np.float32)
    b1v = np.asarray(ln1_b, dtype=np.float32)
    g2 = np.asarray(ln2_g, dtype=np.float32)
    b2v = np.asarray(ln2_b, dtype=np.float32)
    b1f = float(np.asarray(b1, dtype=np.float32).reshape(-1)[0])
    # b2 cancels exactly inside LN2 (constant shift removed by mean
    # subtraction), so it is not passed to the device.

    apply1 = not (np.all(g1 == 1.0) and np.all(b1v == 0.0))
    apply2 = not (np.all(g2 == 1.0) and np.all(b2v == 0.0))

    QT = np.ascontiguousarray(Q.transpose(0, 2, 1))
    KT = np.ascontiguousarray(K.transpose(0, 2, 1))
    W1T = np.ascontiguousarray(W1.T)
    W2T = np.ascontiguousarray(W2.T)

    qmask = (np.arange(QTL)[None, :] < qlen[:, None]).astype(np.float32)
    qsc = (qmask / np.sqrt(np.float32(D))).reshape(B, NQT, P).transpose(0, 2, 1)
    qsc = np.ascontiguousarray(qsc)
    kb = np.where(np.arange(KTL)[None, :] < klen[:, None], 0.0, NEG).astype(
        np.float32
    )
    ident = np.eye(P, dtype=np.float32)
    ones = np.ones((1, P), dtype=np.float32)

    nc = _build(apply1, apply2, b1f)

    in_maps = []
    for c in range(NCORES):
        s = slice(c * BL, (c + 1) * BL)
        m = {
            "QTp": QT[s],
            "KTp": KT[s],
            "Vp": V[s],
            "Qp": Q[s],
            "W1Tp": W1T,
            "W2Tp": W2T,
            "QSCp": qsc[s],
            "KBp": kb[s],
            "IDp": ident,
            "ONEp": ones,
        }
        if apply1:
            m["G1p"] = g1
            m["B1p"] = b1v
        if apply2:
            m["G2p"] = g2
            m["B2p"] = b2v
        in_maps.append(m)

    return nc, in_maps


def kernel(**inputs):
    nc, in_maps = _prepare(**inputs)
    res = run_bass_kernel_spmd(nc, in_maps, list(range(NCORES)))
    out = np.concatenate([res.results[c]["OUTp"] for c in range(NCORES)], axis=0)
    return out.astype(np.float32)



# revision 6
# speedup vs baseline: 1.0962x; 1.0962x over previous
"""Trainium2 Bass kernel for a single-head attention + FFN transformer block.

Math (per batch b), computed in bf16 matmuls with fp32 PSUM accumulation:
  S^T  = K @ Qs^T                  Qs = Q/sqrt(d), invalid-q rows zeroed (host)
  S^T += kbias[k] (x) qmask[q]     rank-1 matmul; kbias = 0 valid / -3e4 masked
  E^T  = exp(S^T)                  invalid-q columns become exactly 1.0, so the
                                   softmax degenerates to uniform over all 512
                                   keys -- matching the reference's fully-masked
                                   row behaviour bit-for-bit in structure.
  rowsum^T[q] = sum_k E^T[k,q]     tiny N=1 matmuls vs ones column
  att  = (E^T' @ V) * recip        E^T used as stationary, [q,d] output
  y    = LN1(Q + att)
  H^T  = relu(W1^T . y^T)          y^T via PE transposes
  Z    = H^T' @ W2^T
  out  = LN2(y + Z)                (+b2 shift cancels inside LN2 exactly)

Sharding: pure data-parallel, 4 batches per core across 8 NeuronCores.
"""

import sys

sys.path.insert(0, "/opt/trn_rl_repo")

import numpy as np
import ml_dtypes

import concourse.bass as bass
import concourse.bacc as bacc
import concourse.mybir as mybir
from concourse import tile
from concourse.bass_utils import run_bass_kernel_spmd

B, QTL, KTL, D = 32, 512, 512, 1024
NCORES = 8
BL = B // NCORES  # batches per core
P = 128
NQT = QTL // P  # 4 q tiles
NKT = KTL // P  # 4 k tiles
NDT = D // P    # 8 d tiles
NCH = D // 512  # 2 free-dim chunks of 512
EPS = 1e-5
NEG = -30000.0
NWARM = 24

F32 = mybir.dt.float32
F32R = mybir.dt.float32r
BF16 = mybir.dt.bfloat16
AF = mybir.ActivationFunctionType
ALU = mybir.AluOpType
BF16NP = ml_dtypes.bfloat16


def _build(apply1: bool, apply2: bool, b1f: float):
    nc = bacc.Bacc(None, target_bir_lowering=False)

    QTd = nc.dram_tensor("QTp", [BL, D, QTL], BF16, kind="ExternalInput")
    KTd = nc.dram_tensor("KTp", [BL, D, KTL], BF16, kind="ExternalInput")
    Vd = nc.dram_tensor("Vp", [BL, KTL, D], BF16, kind="ExternalInput")
    Qd = nc.dram_tensor("Qp", [BL, QTL, D], F32, kind="ExternalInput")
    W1Td = nc.dram_tensor("W1Tp", [D, D], BF16, kind="ExternalInput")
    W2Td = nc.dram_tensor("W2Tp", [D, D], BF16, kind="ExternalInput")
    QMd = nc.dram_tensor("QMp", [BL, QTL], BF16, kind="ExternalInput")
    KBd = nc.dram_tensor("KBp", [BL, KTL], BF16, kind="ExternalInput")
    IDd = nc.dram_tensor("IDp", [P, P], F32, kind="ExternalInput")
    ONEd = nc.dram_tensor("ONEp", [P, 1], BF16, kind="ExternalInput")
    if apply1:
        G1d = nc.dram_tensor("G1p", [D], F32, kind="ExternalInput")
        B1d = nc.dram_tensor("B1p", [D], F32, kind="ExternalInput")
    if apply2:
        G2d = nc.dram_tensor("G2p", [D], F32, kind="ExternalInput")
        B2d = nc.dram_tensor("B2p", [D], F32, kind="ExternalInput")
    OUTd = nc.dram_tensor("OUTp", [BL, QTL, D], F32, kind="ExternalOutput")

    with tile.TileContext(nc) as tc:
        with (
            tc.tile_pool(name="const", bufs=1) as pc,
            tc.tile_pool(name="wts", bufs=1) as pw,
            tc.tile_pool(name="qkin", bufs=2) as pin,
            tc.tile_pool(name="mid", bufs=1) as pmid,
            tc.tile_pool(name="stream", bufs=2) as pst,
            tc.tile_pool(name="small", bufs=2) as psm,
            tc.tile_pool(name="psS", bufs=2, space="PSUM") as psS,
            tc.tile_pool(name="psR", bufs=1, space="PSUM") as psR,
            tc.tile_pool(name="psT", bufs=2, space="PSUM") as psT,
            tc.tile_pool(name="psU", bufs=3, space="PSUM") as psU,
        ):
            for b in range(BL):
                # --- per-batch loads (double-buffered pools) ---
                if b == 0:
                    identr = pc.tile([P, P], F32R, name="identr")
                    nc.sync.dma_start(identr, IDd[:, :].bitcast(F32R))
                QTs = pin.tile([P, NDT, QTL], BF16, tag="qts")
                KTs = pin.tile([P, NDT, KTL], BF16, tag="kts")
                qtr = QTd[b].rearrange("(t p) q -> p t q", p=P)
                ktr = KTd[b].rearrange("(t p) k -> p t k", p=P)
                nch = NDT if b == 0 else 2
                hd = NDT // nch
                for hh in range(nch):
                    sl = slice(hh * hd, (hh + 1) * hd)
                    nc.sync.dma_start(QTs[:, sl, :], qtr[:, sl, :])
                    nc.sync.dma_start(KTs[:, sl, :], ktr[:, sl, :])
                kbr = psm.tile([1, KTL], BF16, tag="kbr")
                nc.sync.dma_start(kbr, KBd[b][None, :])
                qmr = psm.tile([1, QTL], BF16, tag="qmr")
                nc.sync.dma_start(qmr, QMd[b][None, :])
                Vs = pin.tile([P, NKT, D], BF16, tag="vs")
                vr = Vd[b].rearrange("(t p) d -> p t d", p=P)
                nc.sync.dma_start(Vs, vr)
                qps = pst.tile([P, NQT, D], F32, tag="qn", bufs=2)
                qrr = Qd[b].rearrange("(t p) d -> p t d", p=P)
                for hh in range(2):
                    nc.scalar.dma_start(
                        qps[:, 2 * hh : 2 * hh + 2, :], qrr[:, 2 * hh : 2 * hh + 2, :]
                    )
                if b == 0:
                    # one-time constants / weights
                    onesc = pc.tile([P, 1], BF16)
                    nc.sync.dma_start(onesc, ONEd[:, :])
                    epsb = pc.tile([P, 1], F32)
                    nc.vector.memset(epsb, EPS)
                    b1b = pc.tile([P, 1], F32)
                    nc.vector.memset(b1b, b1f)
                    W1Ts = pw.tile([P, NDT, D], BF16)
                    W2Ts = pw.tile([P, NDT, D], BF16)
                    w1r = W1Td.rearrange("(t p) o -> p t o", p=P)
                    w2r = W2Td.rearrange("(t p) o -> p t o", p=P)
                    for hh in range(2):
                        sl = slice(4 * hh, 4 * hh + 4)
                        nc.scalar.dma_start(W1Ts[:, sl, :], w1r[:, sl, :])
                    for hh in range(2):
                        sl = slice(4 * hh, 4 * hh + 4)
                        nc.scalar.dma_start(W2Ts[:, sl, :], w2r[:, sl, :])
                    if apply1:
                        g1t = pc.tile([P, D], F32)
                        nc.gpsimd.dma_start(
                            g1t,
                            bass.AP(tensor=G1d, offset=0, ap=[[0, P], [1, D]]),
                        )
                        b1t = pc.tile([P, D], F32)
                        nc.gpsimd.dma_start(
                            b1t,
                            bass.AP(tensor=B1d, offset=0, ap=[[0, P], [1, D]]),
                        )
                    if apply2:
                        g2t = pc.tile([P, D], F32)
                        nc.gpsimd.dma_start(
                            g2t,
                            bass.AP(tensor=G2d, offset=0, ap=[[0, P], [1, D]]),
                        )
                        b2t = pc.tile([P, D], F32)
                        nc.gpsimd.dma_start(
                            b2t,
                            bass.AP(tensor=B2d, offset=0, ap=[[0, P], [1, D]]),
                        )

                    # Warmup: a short burst of real matmuls so the PE HAM
                    # un-throttles while batch-0's first chunks stream in.
                    wps = psT.tile([P, NKT, P], F32R, tag="tr", name="warm")
                    for _ in range(NWARM):
                        nc.tensor.matmul(
                            wps[:, 0, :].bitcast(F32), identr, identr,
                            start=True, stop=True,
                        )

                ET = pmid.tile([P, NKT, QTL], BF16, tag="eh", bufs=2)

                # --- S^T = K Qs^T, kt-outer so exp overlaps the matmuls ---
                for kt in range(NKT):
                    Sp = psS.tile([P, QTL], F32, tag="s")
                    for dt in range(NDT):
                        nc.tensor.matmul(
                            Sp,
                            KTs[:, dt, kt * P : (kt + 1) * P],
                            QTs[:, dt, :],
                            start=(dt == 0),
                            stop=False,
                        )
                    # rank-1 masked kbias: S^T[k,q] += kbias[k] * qmask[q]
                    nc.tensor.matmul(
                        Sp,
                        kbr[:, kt * P : (kt + 1) * P],
                        qmr[:, :],
                        start=False,
                        stop=True,
                    )
                    nc.scalar.activation(ET[:, kt, :], Sp, AF.Exp)

                # --- rowsum^T[q] = sum_k E^T[k,q] via tiny N=1 matmuls ---
                rsum = psR.tile([P, NQT], F32, tag="rsum")
                for qt in range(NQT):
                    for kt in range(NKT):
                        nc.tensor.matmul(
                            rsum[:, qt : qt + 1],
                            ET[:, kt, qt * P : (qt + 1) * P],
                            onesc,
                            start=(kt == 0),
                            stop=(kt == NKT - 1),
                        )
                recip = psm.tile([P, NQT], F32, tag="recip")
                nc.vector.reciprocal(recip, rsum)

                # --- U = E V, att, residual, LN1 ---
                y = pmid.tile([P, NQT, D], F32R, tag="y")

                def emit_ytr(qt):
                    # y^T: 4 transposes per PSUM bank, 2 copies per q tile.
                    for half in range(2):
                        tps = psT.tile([P, NKT, P], F32R, tag="tr", name="tps")
                        for j in range(4):
                            dt = half * 4 + j
                            nc.tensor.transpose(
                                tps[:, j, :],
                                y[:, qt, dt * P : (dt + 1) * P],
                                identr,
                            )
                        nc.scalar.copy(
                            YT[:, half * 4 : half * 4 + 4, qt * P : (qt + 1) * P],
                            tps,
                        )

                YT = pmid.tile([P, NDT, QTL], BF16, tag="yt")
                for qt in range(NQT):
                    qres = pst.tile([P, D], F32, tag="big4", bufs=3)
                    for ch in range(NCH):
                        Ups = psU.tile([P, 512], F32, tag="u")
                        for kt in range(NKT):
                            nc.tensor.matmul(
                                Ups,
                                ET[:, kt, qt * P : (qt + 1) * P],
                                Vs[:, kt, ch * 512 : (ch + 1) * 512],
                                start=(kt == 0),
                                stop=(kt == NKT - 1),
                            )
                        qch = qres[:, ch * 512 : (ch + 1) * 512]
                        nc.scalar.activation(
                            qch, Ups, AF.Copy, bias=0.0,
                            scale=recip[:, qt : qt + 1],
                        )
                        nc.vector.tensor_add(
                            qch, qch, qps[:, qt, ch * 512 : (ch + 1) * 512]
                        )
                    stats = psm.tile([P, NCH, 6], F32, tag="st1")
                    for ch in range(NCH):
                        nc.vector.bn_stats(
                            stats[:, ch, :], qres[:, ch * 512 : (ch + 1) * 512]
                        )
                    mv = psm.tile([P, 2], F32, tag="mv1")
                    nc.vector.bn_aggr(mv, stats)
                    std = psm.tile([P, 1], F32, tag="std1")
                    nc.scalar.activation(std, mv[:, 1:2], AF.Sqrt, bias=epsb[:, :])
                    nc.vector.reciprocal(std, std)
                    nc.vector.tensor_scalar(
                        y[:, qt, :],
                        qres,
                        scalar1=mv[:, 0:1],
                        scalar2=std,
                        op0=ALU.subtract,
                        op1=ALU.mult,
                    )
                    if apply1:
                        yf = y[:, qt, :].bitcast(F32)
                        nc.vector.tensor_mul(yf, yf, g1t)
                        nc.vector.tensor_add(yf, yf, b1t)
                    if qt >= 1:
                        emit_ytr(qt - 1)
                emit_ytr(NQT - 1)

                # --- FFN1: H^T = relu(sum_d W1T[d,o] yT[d,q]) ---
                HT = pmid.tile([P, NDT, QTL], BF16, tag="ht")
                for ot in range(NDT):
                    Hps = psU.tile([P, QTL], F32, tag="u")
                    for dt in range(NDT):
                        nc.tensor.matmul(
                            Hps,
                            W1Ts[:, dt, ot * P : (ot + 1) * P],
                            YT[:, dt, :],
                            start=(dt == 0),
                            stop=(dt == NDT - 1),
                        )
                    nc.scalar.activation(HT[:, ot, :], Hps, AF.Relu, bias=b1b[:, :])

                # --- FFN2 + LN2 + store ---
                for qt in range(NQT):
                    r2 = pst.tile([P, D], F32, tag="big4", bufs=3)
                    for ch in range(NCH):
                        Zps = psU.tile([P, 512], F32, tag="u")
                        for ot in range(NDT):
                            nc.tensor.matmul(
                                Zps,
                                HT[:, ot, qt * P : (qt + 1) * P],
                                W2Ts[:, ot, ch * 512 : (ch + 1) * 512],
                                start=(ot == 0),
                                stop=(ot == NDT - 1),
                            )
                        nc.vector.tensor_add(
                            r2[:, ch * 512 : (ch + 1) * 512],
                            Zps,
                            y[:, qt, ch * 512 : (ch + 1) * 512].bitcast(F32),
                        )
                    stats2 = psm.tile([P, NCH, 6], F32, tag="st2")
                    for ch in range(NCH):
                        nc.vector.bn_stats(
                            stats2[:, ch, :], r2[:, ch * 512 : (ch + 1) * 512]
                        )
                    mv2 = psm.tile([P, 2], F32, tag="mv2")
                    nc.vector.bn_aggr(mv2, stats2)
                    std2 = psm.tile([P, 1], F32, tag="std2")
                    nc.scalar.activation(std2, mv2[:, 1:2], AF.Sqrt, bias=epsb[:, :])
                    nc.vector.reciprocal(std2, std2)
                    stg = pst.tile([P, D], F32, tag="stg", bufs=2)
                    nc.vector.tensor_scalar(
                        stg,
                        r2,
                        scalar1=mv2[:, 0:1],
                        scalar2=std2,
                        op0=ALU.subtract,
                        op1=ALU.mult,
                    )
                    if apply2:
                        nc.vector.tensor_mul(stg, stg, g2t)
                        nc.vector.tensor_add(stg, stg, b2t)
                    nc.gpsimd.dma_start(
                        OUTd[b].rearrange("(t p) d -> p t d", p=P)[:, qt, :],
                        stg,
                    )

    nc.finalize()
    return nc


def _prepare(Q, K, V, Q_lengths, K_lengths, W1, b1, W2, b2,
             ln1_g, ln1_b, ln2_g, ln2_b):
    Q = np.asarray(Q, dtype=np.float32)
    K = np.asarray(K, dtype=np.float32)
    V = np.asarray(V, dtype=np.float32)
    W1 = np.asarray(W1, dtype=np.float32)
    W2 = np.asarray(W2, dtype=np.float32)
    qlen = np.asarray(Q_lengths).astype(np.int64)
    klen = np.asarray(K_lengths).astype(np.int64)
    g1 = np.asarray(ln1_g, dtype=np.float32)
    b1v = np.asarray(ln1_b, dtype=np.float32)
    g2 = np.asarray(ln2_g, dtype=np.float32)
    b2v = np.asarray(ln2_b, dtype=np.float32)
    b1f = float(np.asarray(b1, dtype=np.float32).reshape(-1)[0])
    # b2 cancels exactly inside LN2 (constant shift removed by mean
    # subtraction), so it is not passed to the device.

    apply1 = not (np.all(g1 == 1.0) and np.all(b1v == 0.0))
    apply2 = not (np.all(g2 == 1.0) and np.all(b2v == 0.0))

    qmask = (np.arange(QTL)[None, :] < qlen[:, None]).astype(np.float32)  # [B,QT]
    kmask = np.arange(KTL)[None, :] < klen[:, None]  # [B,KT]

    # Q^T pre-scaled by 1/sqrt(D), invalid-q rows zeroed -> exp(0)=1 there.
    Qs = Q * (qmask / np.sqrt(np.float32(D)))[:, :, None]
    QT = np.ascontiguousarray(Qs.transpose(0, 2, 1)).astype(BF16NP)
    KT = np.ascontiguousarray(K.transpose(0, 2, 1)).astype(BF16NP)
    Vb = V.astype(BF16NP)
    W1T = np.ascontiguousarray(W1.T).astype(BF16NP)
    W2T = np.ascontiguousarray(W2.T).astype(BF16NP)

    kb = np.where(kmask, 0.0, NEG).astype(BF16NP)  # [B,KT]
    qm = qmask.astype(BF16NP)  # [B,QT]
    ident = np.eye(P, dtype=np.float32)
    ones = np.ones((P, 1), dtype=BF16NP)

    nc = _build(apply1, apply2, b1f)

    in_maps = []
    for c in range(NCORES):
        s = slice(c * BL, (c + 1) * BL)
        m = {
            "QTp": QT[s],
            "KTp": KT[s],
            "Vp": Vb[s],
            "Qp": Q[s],
            "W1Tp": W1T,
            "W2Tp": W2T,
            "QMp": qm[s],
            "KBp": kb[s],
            "IDp": ident,
            "ONEp": ones,
        }
        if apply1:
            m["G1p"] = g1
            m["B1p"] = b1v
        if apply2:
            m["G2p"] = g2
            m["B2p"] = b2v
        in_maps.append(m)

    return nc, in_maps


def kernel(**inputs):
    nc, in_maps = _prepare(**inputs)
    res = run_bass_kernel_spmd(nc, in_maps, list(range(NCORES)))
    out = np.concatenate([res.results[c]["OUTp"] for c in range(NCORES)], axis=0)
    return out.astype(np.float32)
